# revision 24
# baseline (speedup 1.0000x reference)
"""AttentionBlock kernel for Trainium2, 8-core SPMD, fp8 DoubleRow edition.

Problem: x[2,64,64,512] -> GroupNorm(32) -> q,k,v = 1x1 conv -> attention
over the 4096 tokens of each batch image -> out = x + proj(o).

Sharding: 8 cores = 2 batches x 4 query-row blocks of 1024 rows. The host
rolls each core's x so its query block sits at rows [0:1024]; attention is
permutation-invariant over keys. Host pre-casts x and weights to fp8/bf16.

Math restructure vs a direct port (all biases/affine exact):
  - scores^T[j,i] = x_j . R_i with R = diag(s) Wk q^T and q^T built from
    Wq''= diag(s)*Wq*sc (device fold) against raw fp8 x^T. No K tensor is
    ever built (saves a redundant 4096x512x512 matmul per core) and the
    k-bias bk drops entirely (constant per query row -> cancels in softmax,
    as does t.Wk.q).
  - exp uses a global -2 shift to keep e4m3 range; rowsum normalization
    cancels it exactly.
  - Z = P @ x_raw (fp8 DoubleRow); V never materialized:
    attnV_unnorm = Wv^T(s*Z) + rowsum*(t.Wv + bv), the rowsum term rides a
    rank-1 bf16 matmul into the projection PSUM.
  - All heavy matmuls are fp8e4 DoubleRow (2 k-tiles per instruction).
    Scale plan: FW=16 on host weights, FQ=256 Wq fold, qt/R stored x16,
    z stored as s*Z/4, ut = Uu/2, proj psum = 8*Uu@Wp, evac scale 1/(8*rs).
"""
import os
import sys

sys.path.insert(0, "/opt/trn_rl_repo")

import numpy as np
import ml_dtypes

B, H, W_, C = 2, 64, 64, 512
HW = H * W_            # 4096 tokens per batch
GROUPS, GS = 32, 16
EPS = 1e-5
P = 128
CT = C // P            # 4 channel tiles
NKJ = HW // P          # 32 key tiles
NPAIR = NKJ // 2       # 16 DoubleRow key-tile pairs
QBLK = HW // 4         # 1024 query rows per core
SCALE = float(C) ** -0.5
N_QSUB = QBLK // 512   # 2 qi sub-blocks of 512

FW = 16.0              # host weight pre-scale (fp8 range)
FQ = 256.0             # Wq'' fold scale
FQT = 16.0             # qt storage scale
FR = 16.0              # R storage scale
FZ = 0.25              # z storage scale (s*Z/4)
FU = 0.125             # ut storage scale (Uu/8)
FP_PO = FU * FW        # proj psum carries FP_PO * Uu@Wp = 8x
EXP_SHIFT = -2.0

MM_DT_NAME = "fp8dr"

N_WARM = 64            # dummy PE matmuls paced by x chunks (HAM warmth)


def build_kernel():
    import concourse.mybir as mybir
    import concourse.tile as tile
    from concourse import bacc

    f32 = mybir.dt.float32
    bf16 = mybir.dt.bfloat16
    f8 = mybir.dt.float8e4
    DR = mybir.MatmulPerfMode.DoubleRow

    nc = bacc.Bacc("TRN2", target_bir_lowering=False)

    xT8d = nc.dram_tensor("xT8", [C, HW], f8, kind="ExternalInput")
    xn8d = nc.dram_tensor("xn8", [HW, C], f8, kind="ExternalInput")
    xqd = nc.dram_tensor("xq", [QBLK, C], bf16, kind="ExternalInput")
    wkT8d = nc.dram_tensor("WkT8", [C, C], f8, kind="ExternalInput")
    wv8d = nc.dram_tensor("Wv8", [C, C], f8, kind="ExternalInput")
    wp8d = nc.dram_tensor("Wp8", [C, C], f8, kind="ExternalInput")
    wq8rd = nc.dram_tensor("Wq8r", [C, C], f8, kind="ExternalInput")
    bqd = nc.dram_tensor("bq", [1, C], f32, kind="ExternalInput")
    bvd = nc.dram_tensor("bv", [1, C], f32, kind="ExternalInput")
    bpd = nc.dram_tensor("bp", [1, C], f32, kind="ExternalInput")
    gammaT = nc.dram_tensor("gammaT", [C, 1], f32, kind="ExternalInput")
    betaT = nc.dram_tensor("betaT", [C, 1], f32, kind="ExternalInput")
    gseld = nc.dram_tensor("gsel", [C, GROUPS], f32, kind="ExternalInput")
    gexpd = nc.dram_tensor("gexp", [GROUPS, C], f32, kind="ExternalInput")
    ones8d = nc.dram_tensor("ones8", [P, P], f8, kind="ExternalInput")
    outd = nc.dram_tensor("out", [QBLK, C], bf16, kind="ExternalOutput")

    xT8r = xT8d.rearrange("(t p) n -> p t n", p=P)     # [128, 4, 4096]
    xn8r = xn8d.rearrange("(t p) c -> p t c", p=P)     # [128, 32, 512]
    wkT8r = wkT8d.rearrange("(t p) n -> p t n", p=P)
    wv8r = wv8d.rearrange("(t p) n -> p t n", p=P)
    wp8r = wp8d.rearrange("(t p) n -> p t n", p=P)
    wq8rr = wq8rd.rearrange("(t p) n -> p t n", p=P)

    Exp = mybir.ActivationFunctionType.Exp
    Sqrt = mybir.ActivationFunctionType.Sqrt
    Copy = mybir.ActivationFunctionType.Copy
    Ident = mybir.ActivationFunctionType.Identity
    Square = mybir.ActivationFunctionType.Square
    MUL = mybir.AluOpType.mult
    ADD = mybir.AluOpType.add
    SUB = mybir.AluOpType.subtract

    with tile.TileContext(nc) as tc:
        mm = nc.tensor.matmul

        # ---------------- persistent tensors ----------------
        persist = tc.alloc_tile_pool(name="persist", bufs=1)
        xt8 = persist.tile([P, CT, HW], f8, name="xt8")        # x^T fp8
        xn8 = persist.tile([P, NKJ, C], f8, name="xn8")        # x natural fp8
        qt8 = persist.tile([P, CT, QBLK], f8, name="qt8")      # FQT * q^T
        r8 = persist.tile([P, CT, QBLK], f8, name="r8")        # FR * R
        ut8 = persist.tile([P, CT, QBLK], f8, name="ut8")      # FU * Uu^T
        z8 = persist.tile([P, CT, 512], f8, name="z8")         # FZ * s*Z
        wk8 = persist.tile([P, CT, C], f8, name="wk8")         # host FW*Wk^T
        wv8 = persist.tile([P, CT, C], f8, name="wv8")
        wp8 = persist.tile([P, CT, C], f8, name="wp8")
        wq8 = persist.tile([P, CT, C], f8, name="wq8")         # device fold
        onesq8 = persist.tile([P, 8, 16], f8, name="onesq8")   # warm/rowsum lhsT
        c1 = persist.tile([P, 1], f32, name="c1")
        c8 = persist.tile([P, 1], f32, name="c8")
        eps_t = persist.tile([P, 1], f32, name="eps_t")
        gma = persist.tile([P, CT], f32, name="gma")
        bta = persist.tile([P, CT], f32, name="bta")
        gsel_t = persist.tile([P, CT, GROUPS], f32, name="gsel_t")
        gexp_t = persist.tile([GROUPS, CT, P], f32, name="gexp_t")
        st_s = persist.tile([P, CT], f32, name="st_s")         # s = gamma*rstd
        tmm = persist.tile([P, CT], bf16, name="tmm")          # t (bf16)
        foldq = persist.tile([P, CT], f32, name="foldq")       # s*SCALE*FQ
        rcol = persist.tile([P, CT], f32, name="rcol")         # s/FQ... s*FR/(FW*FQT)
        zcol = persist.tile([P, CT], f32, name="zcol")         # s*FZ
        v0col = persist.tile([P, CT], f32, name="v0col")       # FQT*v0
        brow8 = persist.tile([1, C], bf16, name="brow8")       # FP_PO*(bvt@Wp+bp)
        rs_mm = persist.tile([1, QBLK], bf16, name="rs_mm")    # rowsums bf16
        rsr = persist.tile([P, N_QSUB * CT], f32, name="rsr")  # 1/(8*rs) cols
        xres = persist.tile([P, 2 * CT, C], bf16, name="xres")  # residual x rows
        neg2 = persist.tile([P, 1], f32, name="neg2")
        warm_sb = persist.tile([P, 1], f32, name="warm_sb")

        nc.vector.memset(c1, 1.0)
        nc.vector.memset(c8, FP_PO)
        nc.vector.memset(eps_t, EPS)
        nc.vector.memset(neg2, EXP_SHIFT)
        # prewarm ACT tables (order irrelevant; loaded once per func)
        nc.scalar.activation(out=warm_sb, in_=eps_t, func=Exp)
        nc.scalar.activation(out=warm_sb, in_=eps_t, func=Sqrt)
        nc.scalar.activation(out=warm_sb, in_=eps_t, func=Square)

        nc.gpsimd.dma_start(out=onesq8, in_=ones8d[:, :])
        nc.sync.dma_start(out=gma, in_=gammaT.rearrange("(t p) o -> p (t o)", p=P))
        nc.sync.dma_start(out=bta, in_=betaT.rearrange("(t p) o -> p (t o)", p=P))
        nc.sync.dma_start(out=gsel_t, in_=gseld.rearrange("(t p) g -> p t g", p=P))
        nc.sync.dma_start(out=gexp_t, in_=gexpd.rearrange("g (t p) -> g t p", p=P))

        # weight / residual DMAs (gpsimd queue, overlap the xT8 stream)
        wq8r = persist.tile([P, CT, C], f8, name="wq8r")
        bq_row = persist.tile([1, C], f32, name="bq_row")
        bv_row = persist.tile([1, C], f32, name="bv_row")
        bp_row = persist.tile([1, C], f32, name="bp_row")


        # ---------------- stats (+ PE warm dummies paced by chunks) -------
        stats = tc.alloc_tile_pool(name="stats", bufs=1)
        bst = stats.tile([P, CT, 8, 6], f32, name="bst")
        mv = stats.tile([P, CT, 2], f32, name="mv")
        rhs2 = stats.tile([P, CT, 2], f32, name="rhs2")
        gst = stats.tile([GROUPS, 4], f32, name="gst")

        warm_pool = tc.alloc_tile_pool(name="warmp", bufs=1, space="PSUM")
        warm_ps = warm_pool.tile([P, 512], f32, name="warm_ps", tag="warm")
        # xT8 load: 16 parallel 128KB descriptors (sync: ci0/ci1, vector: ci2/ci3)
        for q8 in range(4):
            qsl8 = slice(q8 * 256, (q8 + 1) * 256)
            nc.gpsimd.dma_start(out=xt8[:, 0, qsl8], in_=xT8r[:, 0, qsl8])
        for q4 in range(1, 4):
            qsl4 = slice(q4 * 1024, (q4 + 1) * 1024)
            nc.sync.dma_start(out=xt8[:, 0, qsl4], in_=xT8r[:, 0, qsl4])
        for q4 in range(4):
            qsl4 = slice(q4 * 1024, (q4 + 1) * 1024)
            nc.sync.dma_start(out=xt8[:, 1, qsl4], in_=xT8r[:, 1, qsl4])
        for ci in (2, 3):
            for q4 in range(4):
                qsl4 = slice(q4 * 1024, (q4 + 1) * 1024)
                nc.gpsimd.dma_start(out=xt8[:, ci, qsl4], in_=xT8r[:, ci, qsl4])
        nc.scalar.dma_start(out=wq8r, in_=wq8rr[:, :, :])
        nc.scalar.dma_start(out=wk8, in_=wkT8r[:, :, :])

        sums = stats.tile([P, 2], f32, name="sums")
        sqs = stats.tile([P, 2], f32, name="sqs")
        scr8 = stats.tile([P, 2048], f8, name="scr8")

        # DVE: bn_stats over ci0..ci2 (24 chunks)
        nwarm = 0
        dve_chunks = [(ci, ch) for ci in (0, 3, 1) for ch in range(8)]
        for ci, ch in dve_chunks:
            sl = slice(ch * 512, (ch + 1) * 512)
            nc.vector.bn_stats(out=bst[:, ci, ch, :], in_=xt8[:, ci, sl])
            for r in range(2):
                mm(warm_ps, lhsT=onesq8[:, :, :], rhs=xt8[:, ci, sl],
                   start=(nwarm == 0), stop=(nwarm == 47), skip_group_check=True)
                nwarm += 1
        # ACT: sums/sumsq over ci3 (2 fat 2048-wide pairs)
        for idx in range(2):
            fsl = slice(idx * 2048, (idx + 1) * 2048)
            nc.scalar.activation(out=scr8, in_=xt8[:, 2, fsl], func=Copy,
                                 accum_out=sums[:, idx:idx + 1])
            nc.scalar.activation(out=scr8, in_=xt8[:, 2, fsl], func=Square,
                                 accum_out=sqs[:, idx:idx + 1])

        # gate: release remaining DMA pushes only once ci0 stats are done,
        # so the engine pool serves xT8 first
        gate = stats.tile([1, 6], f32, name="gate")
        nc.gpsimd.dma_start(out=gate, in_=bst[0:1, 0, 7, :])
        for t in range(8):
            nc.gpsimd.dma_start(out=xn8[:, 4 * t:4 * t + 4, :],
                                in_=xn8r[:, 4 * t:4 * t + 4, :])
        nc.gpsimd.dma_start(out=wv8, in_=wv8r[:, :, :])
        nc.gpsimd.dma_start(out=wp8, in_=wp8r[:, :, :])
        nc.gpsimd.dma_start(out=bq_row, in_=bqd[0:1, :])
        nc.gpsimd.dma_start(out=bv_row, in_=bvd[0:1, :])
        nc.gpsimd.dma_start(out=bp_row, in_=bpd[0:1, :])
        xqr = xqd.rearrange("(t p) c -> p t c", p=P)
        for h in range(4):
            nc.gpsimd.dma_start(out=xres[:, 2 * h:2 * h + 2, :],
                                in_=xqr[:, 2 * h:2 * h + 2, :])

        # aggregate: full ci0..ci2 via bn_aggr; ci3 from ACT sums
        wtiny_pool = tc.alloc_tile_pool(name="wtiny", bufs=1, space="PSUM")
        for ci in (0, 3, 1):
            nc.vector.bn_aggr(out=mv[:, ci, :], in_=bst[:, ci, :, :])
            wt = wtiny_pool.tile([1, 2], f32, name="wt", tag="wt")
            mm(wt, lhsT=c1[0:1, 0:1], rhs=mv[0:1, ci, :], skip_group_check=True)
        nc.vector.tensor_copy(rhs2[:, :, 0], mv[:, :, 0])
        nc.vector.tensor_tensor(out=rhs2[:, :, 1], in0=mv[:, :, 0],
                                in1=mv[:, :, 0], op=MUL)
        nc.vector.tensor_tensor(out=rhs2[:, :, 1], in0=rhs2[:, :, 1],
                                in1=mv[:, :, 1], op=ADD)
        for r in range(14):
            mm(warm_ps, lhsT=onesq8[:, :, :], rhs=xt8[:, 0, 0:512],
               start=(r == 0), stop=(r == 13), skip_group_check=True)
        h1m = stats.tile([P, 1], f32, name="h1m")
        nc.vector.tensor_tensor(out=h1m, in0=sums[:, 0:1], in1=sums[:, 1:2],
                                op=ADD)
        nc.vector.tensor_scalar_mul(rhs2[:, 2, 0:1], in0=h1m, scalar1=1.0 / HW)
        nc.vector.tensor_tensor(out=h1m, in0=sqs[:, 0:1], in1=sqs[:, 1:2],
                                op=ADD)
        nc.vector.tensor_scalar_mul(rhs2[:, 2, 1:2], in0=h1m, scalar1=1.0 / HW)
        wt2 = wtiny_pool.tile([1, 2], f32, name="wt2", tag="wt")
        mm(wt2, lhsT=c1[0:1, 0:1], rhs=rhs2[0:1, 2, :], skip_group_check=True)
        nc.scalar.activation(out=warm_sb, in_=warm_ps[:, 0:1], func=Copy)

        smalls = tc.alloc_tile_pool(name="smalls", bufs=1, space="PSUM")
        gs_ps = smalls.tile([GROUPS, 2], f32, name="gs_ps", tag="a")
        for ci in range(CT):
            mm(gs_ps, lhsT=gsel_t[:, ci, :], rhs=rhs2[:, ci, :],
               start=(ci == 0), stop=(ci == CT - 1), skip_group_check=True)
        # gst columns: 0=rstd_g 1=mu_g 2=var_g 3=scratch
        nc.vector.tensor_copy(gst[:, 1:3], gs_ps[:, 0:2])
        nc.vector.tensor_tensor(out=gst[:, 3:4], in0=gst[:, 1:2],
                                in1=gst[:, 1:2], op=MUL)
        nc.vector.tensor_tensor(out=gst[:, 2:3], in0=gst[:, 2:3],
                                in1=gst[:, 3:4], op=SUB)
        nc.scalar.activation(out=gst[:, 3:4], in_=gst[:, 2:3], func=Sqrt,
                             bias=eps_t[0:GROUPS, :], scale=1.0)
        nc.vector.reciprocal(out=gst[:, 0:1], in_=gst[:, 3:4])

        cb_all = smalls.tile([P, CT, 2], f32, name="cb_all", tag="a")
        for ci in range(CT):
            mm(cb_all[:, ci, :], lhsT=gexp_t[:, ci, :], rhs=gst[:, 0:2],
               start=(ci == 0), stop=(ci == CT - 1), skip_group_check=True)
        for r in range(12):
            mm(warm_ps, lhsT=onesq8[:, :, :], rhs=xt8[:, 0, 512:1024],
               start=(r == 0), stop=(r == 11), skip_group_check=True)
        nc.vector.tensor_tensor(out=st_s, in0=cb_all[:, :, 0], in1=gma, op=MUL)
        # t = beta - mu_g * s   (bf16 copy for the bias-chain matmuls)
        tf32 = stats.tile([P, CT], f32, name="tf32")
        nc.vector.tensor_tensor(out=tf32, in0=cb_all[:, :, 1], in1=st_s, op=MUL)
        nc.vector.tensor_tensor(out=tf32, in0=bta, in1=tf32, op=SUB)
        nc.vector.tensor_copy(tmm, tf32)
        # evac scale columns
        nc.vector.tensor_scalar_mul(foldq, in0=st_s, scalar1=SCALE * FQ / FW)
        nc.vector.tensor_scalar_mul(rcol, in0=st_s, scalar1=FR / (FW * FQT))
        nc.vector.tensor_scalar_mul(zcol, in0=st_s, scalar1=FZ)

        # Wq'' fold split across ACT and DVE
        for ci in range(CT):
            if ci % 2 == 0:
                nc.scalar.activation(out=wq8[:, ci, :], in_=wq8r[:, ci, :],
                                     func=Copy, scale=foldq[:, ci:ci + 1])
            else:
                nc.vector.tensor_scalar_mul(wq8[:, ci, :], in0=wq8r[:, ci, :],
                                            scalar1=foldq[:, ci:ci + 1])

        def transpose_row(row_f32, col_ps, rhs_const):
            """[1,512] f32 row -> [128,CT] psum column via tiny fp32 mms."""
            for j in range(CT):
                mm(col_ps[:, j:j + 1], lhsT=row_f32[0:1, j * P:(j + 1) * P],
                   rhs=rhs_const[0:1, 0:1],
                   start=(j == 0), stop=(j == CT - 1), skip_group_check=True)

        # v0 = SCALE*(Wq^T t + bq); store col = FQT*v0
        rowp = smalls.tile([1, C], f32, name="rowp", tag="b")
        row_q = stats.tile([1, C], f32, name="row_q")
        row_v = stats.tile([1, C], f32, name="row_v")
        for ci in range(CT):
            mm(rowp, lhsT=tmm[:, ci:ci + 1], rhs=wq8r[:, ci, :],
               start=(ci == 0), stop=(ci == CT - 1), skip_group_check=True)
        nc.vector.tensor_scalar_mul(row_q, in0=rowp, scalar1=1.0 / FW)
        nc.vector.tensor_tensor(out=row_q, in0=row_q, in1=bq_row,
                                op=ADD)
        nc.vector.tensor_scalar_mul(row_q, in0=row_q, scalar1=SCALE * FQT)
        colp = smalls.tile([P, CT], f32, name="colp", tag="b")
        transpose_row(row_q, colp, c1)
        nc.vector.tensor_copy(v0col, colp)

        # ---------------- qt and R builds (fp8 DR) ----------------
        bld = tc.alloc_tile_pool(name="bld", bufs=3, space="PSUM")
        # qt^T[e, i] = sum_c Wq''[c, e] x^T[c, i]; evac: *1/FQ*FQT + FQT*v0
        for qf in range(N_QSUB):
            for et in range(CT):
                ps = bld.tile([P, 512], f32, name="qtps", tag="bld")
                qsl = slice(qf * 512, (qf + 1) * 512)
                esl = slice(et * P, (et + 1) * P)
                for cp in range(2):
                    mm(ps, lhsT=wq8[:, 2 * cp:2 * cp + 2, esl],
                       rhs=xt8[:, 2 * cp:2 * cp + 2, qsl],
                       start=(cp == 0), stop=(cp == 1),
                       perf_mode=DR, skip_group_check=True)
                nc.scalar.activation(out=qt8[:, et, qsl], in_=ps, func=Ident,
                                     bias=v0col[:, et:et + 1], scale=FQT / FQ)
                del ps
        # R[c, i] = s_c/(FW*FQT)*FR * sum_e WkT[e, c] qt[e, i]
        for qf in range(N_QSUB):
            for ct_ in range(CT):
                ps = bld.tile([P, 512], f32, name="rps", tag="bld")
                qsl = slice(qf * 512, (qf + 1) * 512)
                csl = slice(ct_ * P, (ct_ + 1) * P)
                for ep in range(2):
                    mm(ps, lhsT=wk8[:, 2 * ep:2 * ep + 2, csl],
                       rhs=qt8[:, 2 * ep:2 * ep + 2, qsl],
                       start=(ep == 0), stop=(ep == 1),
                       perf_mode=DR, skip_group_check=True)
                if ct_ % 2 == 0:
                    nc.scalar.activation(out=r8[:, ct_, qsl], in_=ps,
                                         func=Copy, scale=rcol[:, ct_:ct_ + 1])
                else:
                    nc.vector.tensor_scalar_mul(r8[:, ct_, qsl], in0=ps,
                                                scalar1=rcol[:, ct_:ct_ + 1])

        # bvt = t@Wv + bv ; brow8 = FP_PO*(bvt@Wp + bp)
        bvt_ps = smalls.tile([1, C], f32, name="bvt_ps", tag="b")
        for ci in range(CT):
            mm(bvt_ps, lhsT=tmm[:, ci:ci + 1], rhs=wv8[:, ci, :],
               start=(ci == 0), stop=(ci == CT - 1), skip_group_check=True)
        nc.vector.tensor_scalar_mul(row_v, in0=bvt_ps, scalar1=1.0 / FW)
        nc.vector.tensor_tensor(out=row_v, in0=row_v, in1=bv_row,
                                op=ADD)
        bvt_colps = smalls.tile([P, CT], f32, name="bvt_colps", tag="b")
        transpose_row(row_v, bvt_colps, c1)
        bvt_col = stats.tile([P, CT], bf16, name="bvt_col")
        nc.vector.tensor_copy(bvt_col, bvt_colps)
        brow_ps = smalls.tile([1, C], f32, name="brow_ps", tag="b")
        for ci in range(CT):
            mm(brow_ps, lhsT=bvt_col[:, ci:ci + 1], rhs=wp8[:, ci, :],
               start=(ci == 0), stop=(ci == CT - 1), skip_group_check=True)
        browf = stats.tile([1, C], f32, name="browf")
        nc.vector.tensor_scalar_mul(browf, in0=brow_ps, scalar1=1.0 / FW)
        nc.vector.tensor_tensor(out=browf, in0=browf, in1=bp_row,
                                op=ADD)
        nc.vector.tensor_scalar_mul(browf, in0=browf, scalar1=FP_PO)
        nc.vector.tensor_copy(brow8, browf)

        bld.release()
        smalls.release()
        wtiny_pool.release()
        warm_pool.release()

        # ---------------- attention ----------------
        o_ps_pool = tc.alloc_tile_pool(name="o_ps", bufs=1, space="PSUM")
        s_ps_pool = tc.alloc_tile_pool(name="s_ps", bufs=3, space="PSUM")
        rs_ps_pool = tc.alloc_tile_pool(name="rs_ps", bufs=1, space="PSUM")
        pt_pool = tc.alloc_tile_pool(name="pt", bufs=7)
        rssb_pool = tc.alloc_tile_pool(name="rssb", bufs=2)
        out_pool = tc.alloc_tile_pool(name="outp", bufs=3)

        def scores_pair(qb, pr):
            qsl = slice(qb * 512, (qb + 1) * 512)
            pt = pt_pool.tile([P, 2, 512], f8, name="pt", tag="pt")
            for half in range(2):
                kj = 2 * pr + half
                ksl = slice(kj * P, (kj + 1) * P)
                s_ps = s_ps_pool.tile([P, 512], f32, name="s_ps", tag="s")
                for cp in range(2):
                    mm(s_ps, lhsT=xt8[:, 2 * cp:2 * cp + 2, ksl],
                       rhs=r8[:, 2 * cp:2 * cp + 2, qsl],
                       start=(cp == 0), stop=(cp == 1),
                       perf_mode=DR, skip_group_check=True)
                nc.scalar.activation(out=pt[:, half, :], in_=s_ps,
                                     func=Exp, scale=1.0 / FR, bias=neg2)
            return pt

        NPRE = 3  # qb+1 score pairs prefetched into the U/proj bubble
        ptq = {}
        for qb in range(N_QSUB):
            qsl = slice(qb * 512, (qb + 1) * 512)
            z_tiles = [o_ps_pool.tile([P, 512], f32, name=f"o{ci}", tag=f"o{ci}")
                       for ci in range(CT)]
            rs_ps = rs_ps_pool.tile([1, 512], f32, name="rs_ps", tag="rs")

            def accum(pr, pt):
                mm(rs_ps, lhsT=onesq8[:, 0:2, 0:1], rhs=pt[:, :, :],
                   start=(pr == 0), stop=(pr == NPAIR - 1),
                   perf_mode=DR, skip_group_check=True)
                for ci in range(CT):
                    mm(z_tiles[ci],
                       lhsT=xn8[:, 2 * pr:2 * pr + 2, ci * P:(ci + 1) * P],
                       rhs=pt[:, :, :],
                       start=(pr == 0), stop=(pr == NPAIR - 1),
                       perf_mode=DR, skip_group_check=True)

            pt_prev = ptq.pop((qb, 0), None) or scores_pair(qb, 0)
            for pr in range(1, NPAIR):
                pt_cur = ptq.pop((qb, pr), None) or scores_pair(qb, pr)
                accum(pr - 1, pt_prev)
                pt_prev = pt_cur
            accum(NPAIR - 1, pt_prev)
            if qb + 1 < N_QSUB:
                for pr in range(NPRE):
                    ptq[(qb + 1, pr)] = scores_pair(qb + 1, pr)

            # rowsum -> bf16 row + 1/(FP_PO*rs) column
            rs_sb = rssb_pool.tile([1, 512], f32, name="rs_sb", tag="rssb")
            nc.vector.tensor_copy(rs_sb, rs_ps)
            nc.vector.tensor_copy(rs_mm[0:1, qsl], rs_sb)
            rsT_ps = s_ps_pool.tile([P, 512], f32, name="rsT_ps", tag="s")
            transpose_row(rs_sb, rsT_ps[:, 0:CT], c8)
            nc.vector.reciprocal(out=rsr[:, qb * CT:(qb + 1) * CT],
                                 in_=rsT_ps[:, 0:CT])

            # z8 = s*Z/8 (fp8, DVE)
            for ci in range(CT):
                nc.vector.tensor_scalar_mul(z8[:, ci, :], in0=z_tiles[ci],
                                            scalar1=zcol[:, ci:ci + 1])

            # Uu^T = Wv^T (s*Z): psum = FW*FZ*Uu = 4*Uu; store FU*Uu
            for co in range(CT):
                u_ps = o_ps_pool.tile([P, 512], f32, name="u_ps", tag=f"o{co}")
                for cp in range(2):
                    mm(u_ps, lhsT=wv8[:, 2 * cp:2 * cp + 2, co * P:(co + 1) * P],
                       rhs=z8[:, 2 * cp:2 * cp + 2, :],
                       start=(cp == 0), stop=(cp == 1),
                       perf_mode=DR, skip_group_check=True)
                nc.vector.tensor_scalar_mul(ut8[:, co, qsl], in0=u_ps,
                                            scalar1=FU / (FW * FZ))

            # projection: po = FU*FW*(Uu@Wp) + rank-1 rowsum bias
            for jj in range(CT):
                j = qb * CT + jj
                qi0 = j * P
                po = o_ps_pool.tile([P, 512], f32, name="po", tag=f"o{jj}")
                for cp in range(2):
                    mm(po, lhsT=ut8[:, 2 * cp:2 * cp + 2, qi0:qi0 + P],
                       rhs=wp8[:, 2 * cp:2 * cp + 2, :],
                       start=(cp == 0), stop=False,
                       perf_mode=DR, skip_group_check=True)
                mm(po, lhsT=rs_mm[0:1, qi0:qi0 + P], rhs=brow8[0:1, :],
                   start=False, stop=True, skip_group_check=True)
                ot = out_pool.tile([P, 512], bf16, name="ot", tag="ot")
                nc.vector.tensor_scalar_mul(ot, in0=po, scalar1=rsr[:, j:j + 1])
                nc.vector.tensor_tensor(out=ot, in0=ot, in1=xres[:, j, :],
                                        op=ADD)
                nc.sync.dma_start(out=outd[qi0:qi0 + P, 0:256], in_=ot[:, 0:256])
                nc.gpsimd.dma_start(out=outd[qi0:qi0 + P, 256:512],
                                    in_=ot[:, 256:512])

        out_pool.release()
        rssb_pool.release()
        pt_pool.release()
        rs_ps_pool.release()
        s_ps_pool.release()
        o_ps_pool.release()
        stats.release()
        persist.release()

    nc.compile()
    return nc


_GSEL = np.kron(np.eye(GROUPS, dtype=np.float32),
                np.full((GS, 1), 1.0 / GS, np.float32))          # [512, 32]
_GEXP = np.kron(np.eye(GROUPS, dtype=np.float32),
                np.ones((1, GS), np.float32))                    # [32, 512]


def make_in_maps(x, gamma, beta, Wq, bq, Wk, bk, Wv, bv, Wp, bp):
    """Shard FULL inputs into 8 per-core input dicts (host casts fp8/bf16)."""
    f = np.float32
    f8 = ml_dtypes.float8_e4m3
    b16 = ml_dtypes.bfloat16
    x = np.asarray(x, f)
    Wq, Wk, Wv, Wp = (np.asarray(w, f) for w in (Wq, Wk, Wv, Wp))
    common = {
        "WkT8": np.ascontiguousarray(Wk.T * FW).astype(f8),
        "Wv8": (Wv * FW).astype(f8),
        "Wp8": (Wp * FW).astype(f8),
        "Wq8r": (Wq * FW).astype(f8),
        "bq": np.asarray(bq, f).reshape(1, C),
        "bv": np.asarray(bv, f).reshape(1, C),
        "bp": np.asarray(bp, f).reshape(1, C),
        "gammaT": np.asarray(gamma, f).reshape(C, 1),
        "betaT": np.asarray(beta, f).reshape(C, 1),
        "gsel": _GSEL, "gexp": _GEXP,
        "ones8": np.ones((P, P), f8),
    }
    in_maps = []
    for b in range(B):
        xb = x[b].reshape(HW, C)
        for qb in range(4):
            rolled = np.roll(xb, -qb * QBLK, axis=0)
            m = dict(common)
            m["xT8"] = np.ascontiguousarray(rolled.T).astype(f8)
            m["xn8"] = rolled.astype(f8)
            m["xq"] = np.ascontiguousarray(xb[qb * QBLK:(qb + 1) * QBLK]).astype(b16)
            in_maps.append(m)
    return in_maps


def assemble_out(results):
    o = np.empty((B, HW, C), np.float32)
    for b in range(B):
        for qb in range(4):
            o[b, qb * QBLK:(qb + 1) * QBLK] = np.asarray(
                results[b * 4 + qb]["out"]).astype(np.float32)
    return o.reshape(B, H, W_, C)


_NC_CACHE = {}


def run(inputs, trace=False, trace_cores=None):
    from concourse.bass_utils import run_bass_kernel_spmd
    if "nc" not in _NC_CACHE:
        _NC_CACHE["nc"] = build_kernel()
    nc = _NC_CACHE["nc"]
    in_maps = make_in_maps(**inputs)
    res = run_bass_kernel_spmd(nc, in_maps, core_ids=list(range(8)),
                               trace=trace, trace_cores=trace_cores)
    return assemble_out(res.results), res


def kernel(**inputs) -> np.ndarray:
    out, _ = run(inputs, trace=False)
    return out
